# revision 53
# baseline (speedup 1.0000x reference)
"""Trainium2 Bass kernel for nn_Flow (gnn_message_passing).

Strategy
--------
The reference is, per triangle t (200k of them):
    x9   = affine_transform(vertices[t])              # fold into L1 weights
    obj  = MLP_scene(x9)          = L3(relu(L2(relu(L1 x9))))
    feats= [obj, broadcast(pc_emb)]                   # pc_emb same for all t
    flow = MLP_head(feats)        = H3(lrelu(H2(lrelu(H1 feats))))

Host-side folding (all tiny, fp32 numpy):
  * the rigid transform folds into L1:  A1 = M @ W1.T, c1 = c @ W1.T + b1
  * the broadcast pc_emb part of H1 folds into H1's bias (it is constant
    across triangles); pc_emb itself is the scene-MLP of <=6 gathered
    triangles, computed on host.
  * L3 (no activation after it) merges with H1: C = A3 @ B1.

Device (8 cores, data-parallel over triangles, feature-major layout
[features x batch] so no transposes are ever needed):
    h1 = relu(A1.T x + c1)        K=9  ->128
    h2 = relu(A2.T h1 + c2)       K=128->128
    g1 = lrelu(C.T h2 + e)        K=128->64
    g2 = lrelu(B2.T g1 + d2)      K=64 ->64
    fl = B3.T g2                  K=64 ->1    (+d3 on host)
64-wide stages pack two 512-col chunks into the 128 partitions
(PSUM/array quadrants), so every ACT/DVE op runs with all 128 lanes.
The last layer runs transposed (lhsT=g2 block, rhs=B3) so flows land
batch-major as [128,1] PSUM columns in a single accumulator bank —
one PSUM->SBUF copy and one DMA for the whole core (DMA cannot read
PSUM, and [1,N] strip copies would waste 127/128 ACT lanes).
Mask + path-stop postprocessing is O(N) elementwise on host.
"""

import os
from functools import lru_cache

import numpy as np
import ml_dtypes

import concourse.bacc as bacc
import concourse.mybir as mybir
import concourse.tile as tile
from concourse.bass_utils import run_bass_kernel_spmd
from concourse.tile_rust import add_dep_helper

N_TRI = 200000
NCORES = 8
PER_CORE = N_TRI // NCORES          # 25000
CHUNK = 512
PAIR = 2 * CHUNK                    # 1024
NPAIRS = (PER_CORE + PAIR - 1) // PAIR  # 25
NPAD = NPAIRS * PAIR                # 25600
STOPPED = -100.0

# Device dtype for matmul weights/activations: "bf16" or "f32r".
DEV_DT = os.environ.get("FLOW_KERNEL_DT", "bf16")

F32 = mybir.dt.float32


@lru_cache(maxsize=2)
def _build(dev_dt: str):
    DT = mybir.dt.bfloat16 if dev_dt == "bf16" else mybir.dt.float32r
    use_f32r = dev_dt == "f32r"

    nc = bacc.Bacc("TRN2", target_bir_lowering=False, debug=False,
                   num_devices=NCORES)

    # x9 packed for 2-way row-tiled L1: partitions 0:9 = even chunk's
    # features, 32:41 = odd chunk's (both L1 matmuls run concurrently
    # in different PE row groups).
    x9t = nc.dram_tensor("x9t", [18, NPAIRS * CHUNK], DT, kind="ExternalInput")
    wa1 = nc.dram_tensor("wa1", [41, 128], DT, kind="ExternalInput")
    wa2 = nc.dram_tensor("wa2", [128, 128], DT, kind="ExternalInput")
    wc = nc.dram_tensor("wc", [128, 64], DT, kind="ExternalInput")
    wb2 = nc.dram_tensor("wb2", [128, 64], DT, kind="ExternalInput")   # B2 stacked twice
    wb3 = nc.dram_tensor("wb3", [128, 1], DT, kind="ExternalInput")    # B3 stacked twice
    bc1 = nc.dram_tensor("bc1", [128, 1], F32, kind="ExternalInput")
    bc2 = nc.dram_tensor("bc2", [128, 1], F32, kind="ExternalInput")
    bee = nc.dram_tensor("bee", [128, 1], F32, kind="ExternalInput")   # e stacked twice
    bd2 = nc.dram_tensor("bd2", [128, 1], F32, kind="ExternalInput")   # d2 stacked twice
    FCOLS = NPAD // 128                                                # 200
    outd = nc.dram_tensor("flows", [128, FCOLS], F32, kind="ExternalOutput")

    AF = mybir.ActivationFunctionType
    ALU = mybir.AluOpType

    # Explicit PE issue-order chain: keeps concurrent-capable matmul pairs
    # (row/col/quadrant-tiled) adjacent so their streams overlap, instead
    # of letting the scheduler interleave other stages between them.
    _prev_pe = [None]

    _chain = os.environ.get("FLOW_PE_CHAIN", "1") == "1"

    def mm(out_ap, lhsT_ap, rhs_ap):
        inst = nc.tensor.matmul(out_ap, lhsT_ap, rhs_ap)
        if _chain and _prev_pe[0] is not None:
            add_dep_helper(inst.ins, _prev_pe[0].ins, sync=False,
                           reason="pe-order")
        _prev_pe[0] = inst
        return inst

    with tile.TileContext(nc) as tc:
        with (
            tc.tile_pool(name="wpool", bufs=1) as wp,
            tc.tile_pool(name="x9p", bufs=6) as px9,
            tc.tile_pool(name="h1s", bufs=4) as ph1s,
            tc.tile_pool(name="h2s", bufs=4) as ph2s,
            tc.tile_pool(name="g1s", bufs=4) as pg1s,
            tc.tile_pool(name="g2s", bufs=4) as pg2s,
            tc.tile_pool(name="h1p", bufs=3, space="PSUM") as ph1p,
            tc.tile_pool(name="h2p", bufs=2, space="PSUM") as ph2p,
            tc.tile_pool(name="g1p", bufs=1, space="PSUM") as pg1p,
            tc.tile_pool(name="g2p", bufs=1, space="PSUM") as pg2p,
            tc.tile_pool(name="flp", bufs=1, space="PSUM") as pfl,
        ):
            # persistent weights / biases
            wa1_t = wp.tile([41, 128], DT)
            wa2_t = wp.tile([128, 128], DT)
            wc_t = wp.tile([128, 64], DT)
            wb2_t = wp.tile([128, 64], DT)
            wb3_t = wp.tile([128, 1], DT)
            bc1_t = wp.tile([128, 1], F32)
            bc2_t = wp.tile([128, 1], F32)
            bee_t = wp.tile([128, 1], F32)
            bd2_t = wp.tile([128, 1], F32)
            # critical-path weights (first iterations) on the fast HWDGE
            # sync queue (x9(0) is queued first by emit_l1 below); the
            # rest split across gpsimd/scalar queues
            nc.sync.dma_start(wa1_t[:], wa1[:])
            weight_engines = [nc.gpsimd, nc.scalar]
            nc.scalar.dma_start(wa2_t[:], wa2[:])
            for i, (t, d) in enumerate([(bc1_t, bc1), (wc_t, wc), (bc2_t, bc2),
                                        (wb2_t, wb2), (bee_t, bee), (wb3_t, wb3),
                                        (bd2_t, bd2)]):
                weight_engines[i % 2].dma_start(t[:], d[:])

            # batch-major flow accumulator: one column per 128 triangles
            flowp = pfl.tile([128, FCOLS], F32, tag="flowp")
            flows_s = wp.tile([128, FCOLS], F32, tag="flows_s")

            # Fully stage-shifted software pipeline. PE issue order per
            # iteration it:
            #   L1(it) | L2(it-1) | C(it-2) | g2(it-3) | flow(it-4)
            # Every stage consumes transit (ACT/DVE) output produced a full
            # iteration earlier, so the PE never waits on same-iteration
            # activations and concurrency-capable pairs stay adjacent.
            h1s_q, h2s_q, g1s_q, g2s_q = {}, {}, {}, {}

            last = NPAIRS - 1   # its odd chunk is pure padding: skip it

            def emit_l1(p):
                # two 9-row strip DMAs (9KB each) instead of one [41,512]
                # transfer that was 56% zero-padding
                x9 = px9.tile([41, CHUNK], DT, tag="x9")
                nc.sync.dma_start(x9[0:9, :], x9t[0:9, p * CHUNK:(p + 1) * CHUNK])
                h1pe = ph1p.tile([128, CHUNK], F32, tag="h1p")
                mm(h1pe[:], wa1_t[0:9, :], x9[0:9, :])
                h1s = ph1s.tile([128, PAIR], DT, tag="h1s")
                nc.vector.tensor_scalar(h1s[:, 0:CHUNK], h1pe[:],
                                        bc1_t[:, 0:1], 0.0, ALU.add, ALU.max)
                if p != last:
                    nc.sync.dma_start(x9[32:41, :],
                                      x9t[9:18, p * CHUNK:(p + 1) * CHUNK])
                    h1po = ph1p.tile([128, CHUNK], F32, tag="h1p")
                    mm(h1po[:], wa1_t[32:41, :], x9[32:41, :])
                    nc.scalar.activation(h1s[:, CHUNK:PAIR], h1po[:],
                                         AF.Relu, bias=bc1_t[:, 0:1])
                h1s_q[p] = h1s

            def emit_l2(p):
                h1s = h1s_q.pop(p)
                h2pe = ph2p.tile([128, CHUNK], F32, tag="h2p")
                mm(h2pe[:], wa2_t[:], h1s[:, 0:CHUNK])
                h2s = ph2s.tile([128, PAIR], DT, tag="h2s")
                nc.vector.tensor_scalar(h2s[:, 0:CHUNK], h2pe[:],
                                        bc2_t[:, 0:1], 0.0, ALU.add, ALU.max)
                if p != last:
                    h2po = ph2p.tile([128, CHUNK], F32, tag="h2p")
                    mm(h2po[:], wa2_t[:], h1s[:, CHUNK:PAIR])
                    nc.vector.tensor_scalar(h2s[:, CHUNK:PAIR], h2po[:],
                                            bc2_t[:, 0:1], 0.0, ALU.add, ALU.max)
                h2s_q[p] = h2s

            def emit_c(p):
                h2s = h2s_q.pop(p)
                g1p = pg1p.tile([128, CHUNK], F32, tag="g1p")
                mm(g1p[0:64, :], wc_t[:], h2s[:, 0:CHUNK])
                g1s = pg1s.tile([128, CHUNK], DT, tag="g1s")
                if p != last:
                    mm(g1p[64:128, :], wc_t[:], h2s[:, CHUNK:PAIR])
                    nc.scalar.activation(g1s[:], g1p[:], AF.Prelu,
                                         bias=bee_t[:, 0:1], alpha=0.01)
                else:
                    nc.scalar.activation(g1s[0:64, :], g1p[0:64, :], AF.Prelu,
                                         bias=bee_t[0:64, 0:1], alpha=0.01)
                g1s_q[p] = g1s

            def emit_g2(p):
                g1s = g1s_q.pop(p)
                g2p = pg2p.tile([128, CHUNK], F32, tag="g2p")
                mm(g2p[0:64, :], wb2_t[0:64, :], g1s[0:64, :])
                g2s = pg2s.tile([128, CHUNK], DT, tag="g2s")
                if p != last:
                    mm(g2p[64:128, :], wb2_t[64:128, :], g1s[64:128, :])
                    nc.scalar.activation(g2s[:], g2p[:], AF.Prelu,
                                         bias=bd2_t[:, 0:1], alpha=0.01)
                else:
                    nc.scalar.activation(g2s[0:64, :], g2p[0:64, :], AF.Prelu,
                                         bias=bd2_t[0:64, 0:1], alpha=0.01)
                g2s_q[p] = g2s

            def emit_flow(p):
                g2s = g2s_q.pop(p)
                for s in range(1 if p == last else 2):
                    for b in range(4):
                        col = p * 8 + s * 4 + b
                        mm(flowp[:, col:col + 1],
                           g2s[64 * s:64 * s + 64, 128 * b:128 * (b + 1)],
                           wb3_t[64 * s:64 * s + 64, 0:1])

            for it in range(NPAIRS + 4):
                if it < NPAIRS:
                    emit_l1(it)
                if 0 <= it - 1 < NPAIRS:
                    emit_l2(it - 1)
                if 0 <= it - 2 < NPAIRS:
                    emit_c(it - 2)
                if 0 <= it - 3 < NPAIRS:
                    emit_g2(it - 3)
                if 0 <= it - 4 < NPAIRS:
                    emit_flow(it - 4)

            nc.scalar.copy(flows_s[:, 0:FCOLS - 4], flowp[:, 0:FCOLS - 4])
            nc.sync.dma_start(outd[:, 0:FCOLS - 4], flows_s[:, 0:FCOLS - 4])

    nc.compile()
    return nc


def _fold_weights(triangle_vertices, transmitters, receivers,
                  partial_path_candidate, scene_params, head_params):
    tri = np.asarray(triangle_vertices, np.float32)
    tmt = np.asarray(transmitters, np.float32).reshape(3)
    rcv = np.asarray(receivers, np.float32).reshape(3)
    pc = np.asarray(partial_path_candidate).astype(np.int64)
    scene = [(np.asarray(W, np.float32), np.asarray(b, np.float32))
             for W, b in scene_params]
    head = [(np.asarray(W, np.float32), np.asarray(b, np.float32))
            for W, b in head_params]

    # reference: rx = transmitters, tx = receivers, d = rx - tx
    d = tmt - rcv
    scale = np.float32(np.linalg.norm(d))
    u = d / scale
    e = np.zeros(3, np.float32)
    e[np.argmin(np.abs(u))] = 1.0
    v = np.cross(u, e)
    v = v / np.float32(np.linalg.norm(v))
    w = np.cross(u, v)
    B = np.stack([v, w, u]).astype(np.float32)          # (3,3) rotation

    # x9 = raw9 @ M + c
    M = np.zeros((9, 9), np.float32)
    Bt = (B.T / scale).astype(np.float32)
    for i in range(3):
        M[3 * i:3 * i + 3, 3 * i:3 * i + 3] = Bt
    c = np.tile((-rcv / scale) @ B.T, 3).astype(np.float32)

    W1, b1 = scene[0]
    W2, b2 = scene[1]
    W3, b3 = scene[2]
    A1 = (M @ W1.T).astype(np.float32)                  # 9x128
    c1 = (c @ W1.T + b1).astype(np.float32)
    A2 = np.ascontiguousarray(W2.T)                     # 128x128
    c2 = b2
    A3 = np.ascontiguousarray(W3.T)                     # 128x64
    c3 = b3

    # pc_emb: scene-MLP of the <=6 gathered triangles (host, fp32)
    pc_emb = np.zeros((6, 64), np.float32)
    for i, pidx in enumerate(pc):
        if pidx >= 0:
            xg = (((tri[pidx] - rcv) / scale) @ B.T).reshape(1, 9)
            h = np.maximum(xg @ W1.T + b1, 0)
            h = np.maximum(h @ W2.T + b2, 0)
            pc_emb[i] = (h @ W3.T + b3)[0]
    Wh1, bh1 = head[0]
    Wh2, bh2 = head[1]
    Wh3, bh3 = head[2]
    B1 = np.ascontiguousarray(Wh1[:, :64].T)            # 64x64
    d1_eff = (bh1 + pc_emb.reshape(-1) @ Wh1[:, 64:].T).astype(np.float32)

    C = (A3 @ B1).astype(np.float32)                    # 128x64  (L3+H1 merged)
    e_bias = (c3 @ B1 + d1_eff).astype(np.float32)
    B2 = np.ascontiguousarray(Wh2.T)                    # 64x64
    d2 = bh2
    B3 = np.ascontiguousarray(Wh3.T)                    # 64x1
    d3 = np.float32(bh3.reshape(()) if hasattr(bh3, "reshape") else bh3)

    return dict(A1=A1, c1=c1, A2=A2, c2=c2, C=C, e=e_bias, B2=B2, d2=d2,
                B3=B3, d3=d3)


def _run_device(triangle_vertices, folded, trace=False):
    dev_dt = DEV_DT
    nc = _build(dev_dt)
    np_dt = ml_dtypes.bfloat16 if dev_dt == "bf16" else np.float32

    tri = np.asarray(triangle_vertices, np.float32)
    xpad = np.zeros((NCORES, NPAD, 9), np.float32)
    xpad[:, :PER_CORE] = tri.reshape(NCORES, PER_CORE, 9)
    # pack for 2-way row-tiled L1: partitions 0:9 even chunk, 32:41 odd
    w4 = xpad.reshape(NCORES, NPAIRS, 2, CHUNK, 9)
    X = np.zeros((NCORES, 18, NPAIRS * CHUNK), np_dt)
    X[:, 0:9] = w4[:, :, 0].transpose(0, 3, 1, 2).reshape(NCORES, 9, -1)
    X[:, 9:18] = w4[:, :, 1].transpose(0, 3, 1, 2).reshape(NCORES, 9, -1)

    def dup(a):  # stack 64-wide params twice along partitions
        return np.concatenate([a, a], axis=0)

    wa1 = np.zeros((41, 128), np_dt)
    wa1[0:9] = folded["A1"].astype(np_dt)
    wa1[32:41] = folded["A1"].astype(np_dt)
    wa2 = folded["A2"].astype(np_dt)
    wc = folded["C"].astype(np_dt)
    wb2 = dup(folded["B2"]).astype(np_dt)
    wb3 = dup(folded["B3"]).astype(np_dt)
    bc1 = folded["c1"].reshape(128, 1).astype(np.float32)
    bc2 = folded["c2"].reshape(128, 1).astype(np.float32)
    bee = dup(folded["e"].reshape(64, 1)).astype(np.float32)
    bd2 = dup(folded["d2"].reshape(64, 1)).astype(np.float32)

    in_maps = []
    for core in range(NCORES):
        in_maps.append({
            "x9t": np.ascontiguousarray(X[core]),
            "wa1": wa1, "wa2": wa2, "wc": wc, "wb2": wb2, "wb3": wb3,
            "bc1": bc1, "bc2": bc2, "bee": bee, "bd2": bd2,
        })
    bkr = run_bass_kernel_spmd(nc, in_maps, core_ids=list(range(NCORES)),
                               trace=trace)
    flows = np.concatenate(
        [r["flows"].T.reshape(NPAD)[:PER_CORE] for r in bkr.results])
    return flows, bkr


def kernel(triangle_vertices, transmitters, receivers, mask,
           partial_path_candidate, scene_params, head_params):
    folded = _fold_weights(triangle_vertices, transmitters, receivers,
                           partial_path_candidate, scene_params, head_params)
    flows, _ = _run_device(triangle_vertices, folded)
    flows = flows + folded["d3"]

    mask = np.asarray(mask).astype(bool)
    pc = np.asarray(partial_path_candidate).astype(np.int64)
    flows = np.where(mask, flows, np.float32(STOPPED)).astype(np.float32)
    hit = pc == -1
    if hit.any():
        i = int(np.argmax(hit))
        j = i - 1
        last_object = int(pc[max(j, 0)]) if j >= 0 else -1
    else:
        last_object = -1
    if last_object >= 0:
        flows[last_object] = STOPPED
    return flows


# revision 55
# speedup vs baseline: 1.0072x; 1.0072x over previous
"""Trainium2 Bass kernel for nn_Flow (gnn_message_passing).

Strategy
--------
The reference is, per triangle t (200k of them):
    x9   = affine_transform(vertices[t])              # fold into L1 weights
    obj  = MLP_scene(x9)          = L3(relu(L2(relu(L1 x9))))
    feats= [obj, broadcast(pc_emb)]                   # pc_emb same for all t
    flow = MLP_head(feats)        = H3(lrelu(H2(lrelu(H1 feats))))

Host-side folding (all tiny, fp32 numpy):
  * the rigid transform folds into L1:  A1 = M @ W1.T, c1 = c @ W1.T + b1
  * the broadcast pc_emb part of H1 folds into H1's bias (it is constant
    across triangles); pc_emb itself is the scene-MLP of <=6 gathered
    triangles, computed on host.
  * L3 (no activation after it) merges with H1: C = A3 @ B1.

Device (8 cores, data-parallel over triangles, feature-major layout
[features x batch] so no transposes are ever needed):
    h1 = relu(A1.T x + c1)        K=9  ->128
    h2 = relu(A2.T h1 + c2)       K=128->128
    g1 = lrelu(C.T h2 + e)        K=128->64
    g2 = lrelu(B2.T g1 + d2)      K=64 ->64
    fl = B3.T g2                  K=64 ->1    (+d3 on host)
64-wide stages pack two 512-col chunks into the 128 partitions
(PSUM/array quadrants), so every ACT/DVE op runs with all 128 lanes.
The last layer runs transposed (lhsT=g2 block, rhs=B3) so flows land
batch-major as [128,1] PSUM columns in a single accumulator bank —
one PSUM->SBUF copy and one DMA for the whole core (DMA cannot read
PSUM, and [1,N] strip copies would waste 127/128 ACT lanes).
Mask + path-stop postprocessing is O(N) elementwise on host.
"""

import os
from functools import lru_cache

import numpy as np
import ml_dtypes

import concourse.bacc as bacc
import concourse.mybir as mybir
import concourse.tile as tile
from concourse.bass_utils import run_bass_kernel_spmd
from concourse.tile_rust import add_dep_helper

N_TRI = 200000
NCORES = 8
PER_CORE = N_TRI // NCORES          # 25000
CHUNK = 512
PAIR = 2 * CHUNK                    # 1024
NPAIRS = (PER_CORE + PAIR - 1) // PAIR  # 25
NPAD = NPAIRS * PAIR                # 25600
STOPPED = -100.0

# Device dtype for matmul weights/activations: "bf16" or "f32r".
DEV_DT = os.environ.get("FLOW_KERNEL_DT", "bf16")

F32 = mybir.dt.float32


@lru_cache(maxsize=2)
def _build(dev_dt: str):
    DT = mybir.dt.bfloat16 if dev_dt == "bf16" else mybir.dt.float32r
    use_f32r = dev_dt == "f32r"

    nc = bacc.Bacc("TRN2", target_bir_lowering=False, debug=False,
                   num_devices=NCORES)

    # x9 packed for 2-way row-tiled L1: partitions 0:9 = even chunk's
    # features, 32:41 = odd chunk's (both L1 matmuls run concurrently
    # in different PE row groups).
    x9t = nc.dram_tensor("x9t", [18, NPAIRS * CHUNK], DT, kind="ExternalInput")
    wa1 = nc.dram_tensor("wa1", [41, 128], DT, kind="ExternalInput")
    wa2 = nc.dram_tensor("wa2", [128, 128], DT, kind="ExternalInput")
    wc = nc.dram_tensor("wc", [128, 64], DT, kind="ExternalInput")
    wb2 = nc.dram_tensor("wb2", [128, 64], DT, kind="ExternalInput")   # B2 stacked twice
    wb3 = nc.dram_tensor("wb3", [128, 1], DT, kind="ExternalInput")    # B3 stacked twice
    bc1 = nc.dram_tensor("bc1", [128, 1], F32, kind="ExternalInput")
    bc2 = nc.dram_tensor("bc2", [128, 1], F32, kind="ExternalInput")
    bee = nc.dram_tensor("bee", [128, 1], F32, kind="ExternalInput")   # e stacked twice
    bd2 = nc.dram_tensor("bd2", [128, 1], F32, kind="ExternalInput")   # d2 stacked twice
    FCOLS = NPAD // 128                                                # 200
    outd = nc.dram_tensor("flows", [128, FCOLS], F32, kind="ExternalOutput")

    AF = mybir.ActivationFunctionType
    ALU = mybir.AluOpType

    # Explicit PE issue-order chain: keeps concurrent-capable matmul pairs
    # (row/col/quadrant-tiled) adjacent so their streams overlap, instead
    # of letting the scheduler interleave other stages between them.
    _prev_pe = [None]

    _chain = os.environ.get("FLOW_PE_CHAIN", "1") == "1"

    def mm(out_ap, lhsT_ap, rhs_ap):
        inst = nc.tensor.matmul(out_ap, lhsT_ap, rhs_ap)
        if _chain and _prev_pe[0] is not None:
            add_dep_helper(inst.ins, _prev_pe[0].ins, sync=False,
                           reason="pe-order")
        _prev_pe[0] = inst
        return inst

    with tile.TileContext(nc) as tc:
        with (
            tc.tile_pool(name="wpool", bufs=1) as wp,
            tc.tile_pool(name="x9p", bufs=6) as px9,
            tc.tile_pool(name="h1s", bufs=4) as ph1s,
            tc.tile_pool(name="h2s", bufs=4) as ph2s,
            tc.tile_pool(name="g1s", bufs=4) as pg1s,
            tc.tile_pool(name="g2s", bufs=4) as pg2s,
            tc.tile_pool(name="h1p", bufs=3, space="PSUM") as ph1p,
            tc.tile_pool(name="h2p", bufs=2, space="PSUM") as ph2p,
            tc.tile_pool(name="g1p", bufs=1, space="PSUM") as pg1p,
            tc.tile_pool(name="g2p", bufs=1, space="PSUM") as pg2p,
            tc.tile_pool(name="flp", bufs=1, space="PSUM") as pfl,
        ):
            # persistent weights / biases
            wa1_t = wp.tile([41, 128], DT)
            wa2_t = wp.tile([128, 128], DT)
            wc_t = wp.tile([128, 64], DT)
            wb2_t = wp.tile([128, 64], DT)
            wb3_t = wp.tile([128, 1], DT)
            bc1_t = wp.tile([128, 1], F32)
            bc2_t = wp.tile([128, 1], F32)
            bee_t = wp.tile([128, 1], F32)
            bd2_t = wp.tile([128, 1], F32)
            # critical-path weights (first iterations) on the fast HWDGE
            # sync queue (x9(0) is queued first by emit_l1 below); the
            # rest split across gpsimd/scalar queues
            nc.sync.dma_start(wa1_t[:], wa1[:])
            weight_engines = [nc.gpsimd, nc.scalar]
            nc.scalar.dma_start(wa2_t[:], wa2[:])
            for i, (t, d) in enumerate([(bc1_t, bc1), (wc_t, wc), (bc2_t, bc2),
                                        (wb2_t, wb2), (bee_t, bee), (wb3_t, wb3),
                                        (bd2_t, bd2)]):
                weight_engines[i % 2].dma_start(t[:], d[:])

            # batch-major flow accumulator: one column per 128 triangles
            flowp = pfl.tile([128, FCOLS], F32, tag="flowp")
            flows_s = wp.tile([128, FCOLS], F32, tag="flows_s")

            # Fully stage-shifted software pipeline. PE issue order per
            # iteration it:
            #   L1(it) | L2(it-1) | C(it-2) | g2(it-3) | flow(it-4)
            # Every stage consumes transit (ACT/DVE) output produced a full
            # iteration earlier, so the PE never waits on same-iteration
            # activations and concurrency-capable pairs stay adjacent.
            h1s_q, h2s_q, g1s_q, g2s_q = {}, {}, {}, {}

            last = NPAIRS - 1   # its odd chunk is pure padding: skip it

            def emit_l1(p):
                # two 9-row strip DMAs (9KB each) instead of one [41,512]
                # transfer that was 56% zero-padding
                x9 = px9.tile([41, CHUNK], DT, tag="x9")
                nc.sync.dma_start(x9[0:9, :], x9t[0:9, p * CHUNK:(p + 1) * CHUNK])
                h1pe = ph1p.tile([128, CHUNK], F32, tag="h1p")
                mm(h1pe[:], wa1_t[0:9, :], x9[0:9, :])
                h1s = ph1s.tile([128, PAIR], DT, tag="h1s")
                nc.vector.tensor_scalar(h1s[:, 0:CHUNK], h1pe[:],
                                        bc1_t[:, 0:1], 0.0, ALU.add, ALU.max)
                if p != last:
                    nc.sync.dma_start(x9[32:41, :],
                                      x9t[9:18, p * CHUNK:(p + 1) * CHUNK])
                    h1po = ph1p.tile([128, CHUNK], F32, tag="h1p")
                    mm(h1po[:], wa1_t[32:41, :], x9[32:41, :])
                    nc.scalar.activation(h1s[:, CHUNK:PAIR], h1po[:],
                                         AF.Relu, bias=bc1_t[:, 0:1])
                h1s_q[p] = h1s

            def emit_l2(p):
                h1s = h1s_q.pop(p)
                h2pe = ph2p.tile([128, CHUNK], F32, tag="h2p")
                mm(h2pe[:], wa2_t[:], h1s[:, 0:CHUNK])
                h2s = ph2s.tile([128, PAIR], DT, tag="h2s")
                nc.vector.tensor_scalar(h2s[:, 0:CHUNK], h2pe[:],
                                        bc2_t[:, 0:1], 0.0, ALU.add, ALU.max)
                if p != last:
                    h2po = ph2p.tile([128, CHUNK], F32, tag="h2p")
                    mm(h2po[:], wa2_t[:], h1s[:, CHUNK:PAIR])
                    nc.vector.tensor_scalar(h2s[:, CHUNK:PAIR], h2po[:],
                                            bc2_t[:, 0:1], 0.0, ALU.add, ALU.max)
                h2s_q[p] = h2s

            def emit_c(p):
                h2s = h2s_q.pop(p)
                g1p = pg1p.tile([128, CHUNK], F32, tag="g1p")
                mm(g1p[0:64, :], wc_t[:], h2s[:, 0:CHUNK])
                g1s = pg1s.tile([128, CHUNK], DT, tag="g1s")
                if p != last:
                    mm(g1p[64:128, :], wc_t[:], h2s[:, CHUNK:PAIR])
                    nc.scalar.activation(g1s[:], g1p[:], AF.Prelu,
                                         bias=bee_t[:, 0:1], alpha=0.01)
                else:
                    nc.scalar.activation(g1s[0:64, :], g1p[0:64, :], AF.Prelu,
                                         bias=bee_t[0:64, 0:1], alpha=0.01)
                g1s_q[p] = g1s

            def emit_g2(p):
                g1s = g1s_q.pop(p)
                g2p = pg2p.tile([128, CHUNK], F32, tag="g2p")
                mm(g2p[0:64, :], wb2_t[0:64, :], g1s[0:64, :])
                g2s = pg2s.tile([128, CHUNK], DT, tag="g2s")
                if p != last:
                    mm(g2p[64:128, :], wb2_t[64:128, :], g1s[64:128, :])
                    nc.scalar.activation(g2s[:], g2p[:], AF.Prelu,
                                         bias=bd2_t[:, 0:1], alpha=0.01)
                else:
                    nc.scalar.activation(g2s[0:64, :], g2p[0:64, :], AF.Prelu,
                                         bias=bd2_t[0:64, 0:1], alpha=0.01)
                g2s_q[p] = g2s

            def emit_flow(p):
                g2s = g2s_q.pop(p)
                for s in range(1 if p == last else 2):
                    for b in range(4):
                        col = p * 8 + s * 4 + b
                        mm(flowp[:, col:col + 1],
                           g2s[64 * s:64 * s + 64, 128 * b:128 * (b + 1)],
                           wb3_t[64 * s:64 * s + 64, 0:1])

            for it in range(NPAIRS + 4):
                if it < NPAIRS:
                    emit_l1(it)
                if 0 <= it - 1 < NPAIRS:
                    emit_l2(it - 1)
                if 0 <= it - 2 < NPAIRS:
                    emit_c(it - 2)
                if 0 <= it - 3 < NPAIRS:
                    emit_g2(it - 3)
                if 0 <= it - 4 < NPAIRS:
                    emit_flow(it - 4)

            nc.scalar.copy(flows_s[:, 0:FCOLS - 4], flowp[:, 0:FCOLS - 4])
            nc.sync.dma_start(outd[:, 0:FCOLS - 4], flows_s[:, 0:FCOLS - 4])

    nc.compile()
    return nc


def _fold_weights(triangle_vertices, transmitters, receivers,
                  partial_path_candidate, scene_params, head_params):
    tri = np.asarray(triangle_vertices, np.float32)
    tmt = np.asarray(transmitters, np.float32).reshape(3)
    rcv = np.asarray(receivers, np.float32).reshape(3)
    pc = np.asarray(partial_path_candidate).astype(np.int64)
    scene = [(np.asarray(W, np.float32), np.asarray(b, np.float32))
             for W, b in scene_params]
    head = [(np.asarray(W, np.float32), np.asarray(b, np.float32))
            for W, b in head_params]

    # reference: rx = transmitters, tx = receivers, d = rx - tx
    d = tmt - rcv
    scale = np.float32(np.linalg.norm(d))
    u = d / scale
    e = np.zeros(3, np.float32)
    e[np.argmin(np.abs(u))] = 1.0
    v = np.cross(u, e)
    v = v / np.float32(np.linalg.norm(v))
    w = np.cross(u, v)
    B = np.stack([v, w, u]).astype(np.float32)          # (3,3) rotation

    # x9 = raw9 @ M + c
    M = np.zeros((9, 9), np.float32)
    Bt = (B.T / scale).astype(np.float32)
    for i in range(3):
        M[3 * i:3 * i + 3, 3 * i:3 * i + 3] = Bt
    c = np.tile((-rcv / scale) @ B.T, 3).astype(np.float32)

    W1, b1 = scene[0]
    W2, b2 = scene[1]
    W3, b3 = scene[2]
    A1 = (M @ W1.T).astype(np.float32)                  # 9x128
    c1 = (c @ W1.T + b1).astype(np.float32)
    A2 = np.ascontiguousarray(W2.T)                     # 128x128
    c2 = b2
    A3 = np.ascontiguousarray(W3.T)                     # 128x64
    c3 = b3

    # pc_emb: scene-MLP of the <=6 gathered triangles (host, fp32)
    pc_emb = np.zeros((6, 64), np.float32)
    for i, pidx in enumerate(pc):
        if pidx >= 0:
            xg = (((tri[pidx] - rcv) / scale) @ B.T).reshape(1, 9)
            h = np.maximum(xg @ W1.T + b1, 0)
            h = np.maximum(h @ W2.T + b2, 0)
            pc_emb[i] = (h @ W3.T + b3)[0]
    Wh1, bh1 = head[0]
    Wh2, bh2 = head[1]
    Wh3, bh3 = head[2]
    B1 = np.ascontiguousarray(Wh1[:, :64].T)            # 64x64
    d1_eff = (bh1 + pc_emb.reshape(-1) @ Wh1[:, 64:].T).astype(np.float32)

    C = (A3 @ B1).astype(np.float32)                    # 128x64  (L3+H1 merged)
    e_bias = (c3 @ B1 + d1_eff).astype(np.float32)
    B2 = np.ascontiguousarray(Wh2.T)                    # 64x64
    d2 = bh2
    B3 = np.ascontiguousarray(Wh3.T)                    # 64x1
    d3 = np.float32(bh3.reshape(()) if hasattr(bh3, "reshape") else bh3)

    return dict(A1=A1, c1=c1, A2=A2, c2=c2, C=C, e=e_bias, B2=B2, d2=d2,
                B3=B3, d3=d3)


def _run_device(triangle_vertices, folded, trace=False):
    dev_dt = DEV_DT
    nc = _build(dev_dt)
    np_dt = ml_dtypes.bfloat16 if dev_dt == "bf16" else np.float32

    tri = np.asarray(triangle_vertices, np.float32)
    xpad = np.zeros((NCORES, NPAD, 9), np.float32)
    xpad[:, :PER_CORE] = tri.reshape(NCORES, PER_CORE, 9)
    # pack for 2-way row-tiled L1: partitions 0:9 even chunk, 32:41 odd
    w4 = xpad.reshape(NCORES, NPAIRS, 2, CHUNK, 9)
    X = np.zeros((NCORES, 18, NPAIRS * CHUNK), np_dt)
    X[:, 0:9] = w4[:, :, 0].transpose(0, 3, 1, 2).reshape(NCORES, 9, -1)
    X[:, 9:18] = w4[:, :, 1].transpose(0, 3, 1, 2).reshape(NCORES, 9, -1)

    def dup(a):  # stack 64-wide params twice along partitions
        return np.concatenate([a, a], axis=0)

    wa1 = np.zeros((41, 128), np_dt)
    wa1[0:9] = folded["A1"].astype(np_dt)
    wa1[32:41] = folded["A1"].astype(np_dt)
    wa2 = folded["A2"].astype(np_dt)
    wc = folded["C"].astype(np_dt)
    wb2 = dup(folded["B2"]).astype(np_dt)
    wb3 = dup(folded["B3"]).astype(np_dt)
    bc1 = folded["c1"].reshape(128, 1).astype(np.float32)
    bc2 = folded["c2"].reshape(128, 1).astype(np.float32)
    bee = dup(folded["e"].reshape(64, 1)).astype(np.float32)
    bd2 = dup(folded["d2"].reshape(64, 1)).astype(np.float32)

    in_maps = []
    for core in range(NCORES):
        in_maps.append({
            "x9t": np.ascontiguousarray(X[core]),
            "wa1": wa1, "wa2": wa2, "wc": wc, "wb2": wb2, "wb3": wb3,
            "bc1": bc1, "bc2": bc2, "bee": bee, "bd2": bd2,
        })
    bkr = run_bass_kernel_spmd(nc, in_maps, core_ids=list(range(NCORES)),
                               trace=trace)
    flows = np.concatenate(
        [r["flows"].T.reshape(NPAD)[:PER_CORE] for r in bkr.results])
    return flows, bkr


def kernel(triangle_vertices, transmitters, receivers, mask,
           partial_path_candidate, scene_params, head_params):
    folded = _fold_weights(triangle_vertices, transmitters, receivers,
                           partial_path_candidate, scene_params, head_params)
    flows, _ = _run_device(triangle_vertices, folded)
    flows = flows + folded["d3"]

    mask = np.asarray(mask).astype(bool)
    pc = np.asarray(partial_path_candidate).astype(np.int64)
    flows = np.where(mask, flows, np.float32(STOPPED)).astype(np.float32)
    hit = pc == -1
    if hit.any():
        i = int(np.argmax(hit))
        j = i - 1
        last_object = int(pc[max(j, 0)]) if j >= 0 else -1
    else:
        last_object = -1
    if last_object >= 0:
        flows[last_object] = STOPPED
    return flows
